# revision 1
# baseline (speedup 1.0000x reference)
"""Trainium2 Bass kernel for nn_ChamferLoss (reflection-symmetry chamfer loss).

Sharding: pure data parallel - batch b -> core b (B=8, 8 cores). Each core
computes its batch's bidirectional chamfer sums over 3 reflection heads plus
the orthogonality regularizer; the host sums the 8 scalar partials.

Distance matmul: d[i,j] = |x_i|^2 + |y_j|^2 - 2 x_i.y_j on the PE with fp32
operands decomposed into 3 bf16 levels (hh,hm,hl,mh,mm,lh cross terms)
stacked along K=24 -> full-speed bf16 matmul at fp32-grade accuracy (~1e-6
abs on d). fp32 matmul itself runs at 1/4 rate, hence the split trick.
Both chamfer directions are computed with their own matmuls (d and d^T) so
every min is a free-dim reduction. Reflected points never touch DRAM: the
reflection, |y|^2 (= |x|^2 + 4*s*off), the bf16 splits, and the K-stacked
operand tensors are all built on-chip in a [128, rows, tiles] layout and
PE-transposed into matmul form.

Min consumption (tensor_tensor_reduce hangs this runtime's DVE, so it is
not used): per 128-row block, 4 PSUM stripes of 1024 distances are reduced
by a mix of two flavors - fp16-tree blocks (ACT evacuates stripes to SBUF
fp16, DVE runs a 2x-mode TT-min tree; fp16 rounding only perturbs selected
minima by ~d*2^-11) and fp32-chain blocks (DVE TT-mins straight from PSUM,
no ACT traffic). A 3/7 evenly-spread chain fraction balances ACT vs DVE;
cost-model timeline: ~721 us/core, DVE 93% busy.
"""

import sys

sys.path.insert(0, "/opt/trn_rl_repo")

from contextlib import ExitStack

import numpy as np

import concourse.bass as bass
import concourse.bacc as bacc
import concourse.tile as tile
from concourse import mybir
from concourse.masks import make_identity
from concourse.bass_utils import run_bass_kernel_spmd

F32 = mybir.dt.float32
BF16 = mybir.dt.bfloat16
FP16 = mybir.dt.float16
AX = mybir.AxisListType
OP = mybir.AluOpType
AF = mybir.ActivationFunctionType

P = 128
H = 3
REG_COEF = 25.0
B = 8

# level patterns for the 6 kept cross products (x-level, y-level):
# (h,h) (h,m) (h,l) (m,h) (m,m) (l,h)
L_LEVELS = [0, 0, 0, 1, 1, 2]  # stationary-side level per 3-row group
R_LEVELS = [0, 1, 2, 0, 1, 0]  # moving-side level per 3-row group


import os
ASM_ENGINE = os.environ.get("CHAMFER_ASM_ENGINE", "gpsimd")


def ASM(nc):
    return getattr(nc, ASM_ENGINE)


def _split3(nc, pool, src, shape, tag):
    """3-level bf16 split of an f32 tile: src ~= b0+b1+b2 (rel ~2^-25)."""
    outs = []
    cur = src
    for lv in range(3):
        b = pool.tile(shape, BF16, tag=f"{tag}b{lv}")
        nc.scalar.copy(out=b, in_=cur)
        outs.append(b)
        if lv < 2:
            r = pool.tile(shape, F32, tag=f"{tag}r{lv}")
            nc.vector.tensor_tensor(out=r, in0=cur, in1=b, op=OP.subtract)
            cur = r
    return outs


def emit_chamfer(nc, n=4096):
    NT = n // P           # number of 128-point blocks
    W = min(1024, n)      # psum stripe width (free dim)
    NST = n // W          # stripes per row-block
    NMM = W // 512        # matmuls per stripe

    pts = nc.dram_tensor("pts", [n, 3], F32, kind="ExternalInput").ap()
    yp = nc.dram_tensor("yp", [H, 4], F32, kind="ExternalInput").ap()
    out = nc.dram_tensor("out", [1, 1], F32, kind="ExternalOutput").ap()

    with ExitStack() as ctx:
        tc = ctx.enter_context(tile.TileContext(nc))
        const = ctx.enter_context(tc.tile_pool(name="const", bufs=1))
        work = ctx.enter_context(tc.tile_pool(name="work", bufs=4))
        headp = ctx.enter_context(tc.tile_pool(name="headp", bufs=2))
        pstripe = ctx.enter_context(tc.tile_pool(
            name="pstripe", bufs=4, space="PSUM"))

        id128 = const.tile([P, P], BF16)
        make_identity(nc, id128)

        # ---- load points: Xn[p, t, c] = pts[t*128+p, c]
        Xn = const.tile([P, NT, 3], F32)
        nc.sync.dma_start(out=Xn, in_=pts.rearrange("(t p) c -> p t c", p=P))

        # ---- yp broadcast to all partitions: ypb[p, h, k] = yp[h, k]
        ypb = const.tile([P, H, 4], F32)
        yp_b = bass.AP(tensor=yp.tensor, offset=yp.offset,
                       ap=[[0, P], [4, H], [1, 4]])
        nc.sync.dma_start(out=ypb, in_=yp_b)

        # ---- sx = |x|^2 per point, in [128, NT] layout
        Xsq = work.tile([P, NT, 3], F32)
        nc.scalar.activation(out=Xsq, in_=Xn, func=AF.Square)
        sx = const.tile([P, NT], F32)
        nc.vector.tensor_tensor(out=sx, in0=Xsq[:, :, 0], in1=Xsq[:, :, 1], op=OP.add)
        nc.vector.tensor_tensor(out=sx, in0=sx, in1=Xsq[:, :, 2], op=OP.add)

        # ---- u = -2x and its bf16 splits; sx splits
        U = work.tile([P, NT, 3], F32)
        nc.scalar.mul(out=U, in_=Xn, mul=-2.0)
        ub = _split3(nc, work, U, [P, NT, 3], "u")
        sxb = _split3(nc, work, sx, [P, NT], "sx")

        # ---- build stacked X-side aug tile [128, 64, NT] (bf16), then
        # transpose to XS [64, n] (matmul base partitions must be 0/32):
        #   rows 0-23  = dir-1 lhsT: u groups L_LEVELS, sx splits, ones
        #   rows 32-55 = dir-2 rhs : u groups R_LEVELS, ones, sx splits
        XSa = work.tile([P, 64, NT], BF16)
        ASM(nc).memset(XSa[:, 24:32, :], 0.0)
        ASM(nc).memset(XSa[:, 56:64, :], 0.0)
        for g, lv in enumerate(L_LEVELS):
            ASM(nc).tensor_copy(out=XSa[:, 3 * g:3 * g + 3, :],
                                  in_=ub[lv].rearrange("p t c -> p c t"))
        for l in range(3):
            ASM(nc).tensor_copy(out=XSa[:, 18 + l, :], in_=sxb[l])
        ASM(nc).memset(XSa[:, 21:24, :], 1.0)
        for g, lv in enumerate(R_LEVELS):
            ASM(nc).tensor_copy(out=XSa[:, 32 + 3 * g:32 + 3 * g + 3, :],
                                  in_=ub[lv].rearrange("p t c -> p c t"))
        ASM(nc).memset(XSa[:, 50:53, :], 1.0)
        for l in range(3):
            ASM(nc).tensor_copy(out=XSa[:, 53 + l, :], in_=sxb[l])

        # per-chunk tiles so matmuls only wait on the chunk they read
        XS = []
        for g in range(NST):
            pt = pstripe.tile([64, W], BF16, tag="stripe")
            for k in range(W // P):
                t = g * (W // P) + k
                nc.tensor.transpose(pt[:, k * P:(k + 1) * P], XSa[:, :, t], id128)
            xc = const.tile([64, W], BF16, tag=f"XSc{g}")
            nc.scalar.copy(out=xc, in_=pt)
            XS.append(xc)

        # ---- collected mins: [128, 2*NT*H] (dir1 + dir2 per head)
        mins_all = const.tile([P, 2 * NT * H], F32)
        # normalized normals per head (redundant across partitions)
        nhat = const.tile([P, H, 3], F32)

        for h in range(H):
            # --- normalize head normal (per-partition redundant, exact DVE ops)
            sqn = headp.tile([P, 3], F32, tag="sqn")
            nc.vector.tensor_tensor(out=sqn, in0=ypb[:, h, 0:3], in1=ypb[:, h, 0:3],
                                    op=OP.mult)
            nn = headp.tile([P, 1], F32, tag="nn")
            nc.vector.tensor_reduce(out=nn, in_=sqn, axis=AX.X, op=OP.add)
            sq_ = headp.tile([P, 1], F32, tag="sq_")
            nc.scalar.activation(out=sq_, in_=nn, func=AF.Sqrt)
            rs0 = headp.tile([P, 1], F32, tag="rs0")
            nc.vector.reciprocal(out=rs0, in_=sq_)
            # one Newton step: rs = rs0*(1.5 - 0.5*nn*rs0^2)
            a = headp.tile([P, 1], F32, tag="nta")
            nc.vector.tensor_tensor(out=a, in0=rs0, in1=rs0, op=OP.mult)
            nc.vector.tensor_tensor(out=a, in0=a, in1=nn, op=OP.mult)
            nc.vector.tensor_scalar(out=a, in0=a, scalar1=-0.5, scalar2=1.5,
                                    op0=OP.mult, op1=OP.add)
            rs = headp.tile([P, 1], F32, tag="rs")
            nc.vector.tensor_tensor(out=rs, in0=rs0, in1=a, op=OP.mult)
            nc.vector.tensor_scalar(out=nhat[:, h, :], in0=ypb[:, h, 0:3], scalar1=rs,
                                    scalar2=None, op0=OP.mult)
            off = ypb[:, h, 3:4]

            # --- s[p,t] = nhat . x + off   (signed plane distance)
            s = headp.tile([P, NT], F32, tag="s")
            t0 = headp.tile([P, NT], F32, tag="t0")
            nc.vector.tensor_scalar(out=s, in0=Xn[:, :, 0], scalar1=nhat[:, h, 0:1],
                                    scalar2=off, op0=OP.mult, op1=OP.add)
            nc.vector.tensor_scalar(out=t0, in0=Xn[:, :, 1], scalar1=nhat[:, h, 1:2],
                                    scalar2=None, op0=OP.mult)
            nc.vector.tensor_tensor(out=s, in0=s, in1=t0, op=OP.add)
            nc.vector.tensor_scalar(out=t0, in0=Xn[:, :, 2], scalar1=nhat[:, h, 2:3],
                                    scalar2=None, op0=OP.mult)
            nc.vector.tensor_tensor(out=s, in0=s, in1=t0, op=OP.add)

            # --- reflected points Yn = x - 2 s nhat ; sy = sx + 4*off*s
            m2 = headp.tile([P, 3], F32, tag="m2")
            nc.vector.tensor_scalar(out=m2, in0=nhat[:, h, :], scalar1=-2.0,
                                    scalar2=None, op0=OP.mult)
            Yn = headp.tile([P, NT, 3], F32, tag="Yn")
            tc_ = headp.tile([P, NT], F32, tag="tc_")
            for c in range(3):
                nc.vector.tensor_scalar(out=tc_, in0=s, scalar1=m2[:, c:c + 1],
                                        scalar2=None, op0=OP.mult)
                nc.vector.tensor_tensor(out=Yn[:, :, c], in0=Xn[:, :, c], in1=tc_,
                                        op=OP.add)
            o4 = headp.tile([P, 1], F32, tag="o4")
            nc.vector.tensor_scalar(out=o4, in0=off, scalar1=4.0, scalar2=None,
                                    op0=OP.mult)
            sy = headp.tile([P, NT], F32, tag="sy")
            nc.vector.tensor_scalar(out=sy, in0=s, scalar1=o4, scalar2=None,
                                    op0=OP.mult)
            nc.vector.tensor_tensor(out=sy, in0=sy, in1=sx, op=OP.add)

            # --- y / sy splits and stacked Y-side aug tile
            yb = _split3(nc, headp, Yn, [P, NT, 3], "y")
            syb = _split3(nc, headp, sy, [P, NT], "sy")
            YSa = headp.tile([P, 64, NT], BF16, tag="YSa")
            ASM(nc).memset(YSa[:, 24:32, :], 0.0)
            ASM(nc).memset(YSa[:, 56:64, :], 0.0)
            # rows 0-23 = dir-1 rhs: y groups R_LEVELS, ones, sy splits
            for g, lv in enumerate(R_LEVELS):
                ASM(nc).tensor_copy(out=YSa[:, 3 * g:3 * g + 3, :],
                                      in_=yb[lv].rearrange("p t c -> p c t"))
            ASM(nc).memset(YSa[:, 18:21, :], 1.0)
            for l in range(3):
                ASM(nc).tensor_copy(out=YSa[:, 21 + l, :], in_=syb[l])
            # rows 32-55 = dir-2 lhsT: y groups L_LEVELS, sy splits, ones
            for g, lv in enumerate(L_LEVELS):
                ASM(nc).tensor_copy(out=YSa[:, 32 + 3 * g:32 + 3 * g + 3, :],
                                      in_=yb[lv].rearrange("p t c -> p c t"))
            for l in range(3):
                ASM(nc).tensor_copy(out=YSa[:, 50 + l, :], in_=syb[l])
            ASM(nc).memset(YSa[:, 53:56, :], 1.0)

            YS = []
            for g in range(NST):
                pt = pstripe.tile([64, W], BF16, tag="stripe")
                for k in range(W // P):
                    t = g * (W // P) + k
                    nc.tensor.transpose(pt[:, k * P:(k + 1) * P], YSa[:, :, t], id128)
                yc = headp.tile([64, W], BF16, tag=f"YSc{g}")
                nc.scalar.copy(out=yc, in_=pt)
                YS.append(yc)

            # --- main loops, both directions interleaved per 128-row
            # block. Two block flavors:
            #  fp16-tree: ACT evacuates all stripes PSUM->SBUF fp16; DVE
            #    does a 2x-mode fp16 TT-min tree + one fp16 reduce_min.
            #  fp32-chain: DVE TT-mins PSUM stripes into a running SBUF min
            #    (no ACT work) + one fp32 reduce_min.
            def emit_block_f16(d2, i):
                if d2 == 0:
                    LT, RT, lo = XS, YS, 0
                else:
                    LT, RT, lo = YS, XS, 32
                lc, lof = (i * P) // W, (i * P) % W
                lhsT = LT[lc][lo:lo + 24, lof:lof + P]
                rowcol = mins_all[:, h * 2 * NT + d2 * NT + i:
                                  h * 2 * NT + d2 * NT + i + 1]
                sb = []
                for g in range(NST):
                    ps = pstripe.tile([P, W], F32, tag="stripe")
                    for m in range(NMM):
                        nc.tensor.matmul(
                            ps[:, m * 512:(m + 1) * 512],
                            lhsT=lhsT,
                            rhs=RT[g][lo:lo + 24,
                                      m * 512:(m + 1) * 512],
                            start=True, stop=True)
                    s16 = work.tile([P, W], FP16, tag=f"f16s{g % 4}")
                    nc.scalar.copy(out=s16, in_=ps)
                    sb.append(s16)
                m0 = work.tile([P, W], FP16, tag="f16m0")
                nc.vector.tensor_tensor(out=m0, in0=sb[0], in1=sb[1], op=OP.min)
                if NST >= 4:
                    m1 = work.tile([P, W], FP16, tag="f16m1")
                    nc.vector.tensor_tensor(out=m1, in0=sb[2], in1=sb[3],
                                            op=OP.min)
                    nc.vector.tensor_tensor(out=m0, in0=m0, in1=m1, op=OP.min)
                # narrow with 2x-mode fp16 TT halvings before the 1x reduce
                n1 = work.tile([P, W // 2], FP16, tag="f16n1")
                nc.vector.tensor_tensor(out=n1, in0=m0[:, 0:W // 2],
                                        in1=m0[:, W // 2:W], op=OP.min)
                n2 = work.tile([P, W // 4], FP16, tag="f16n2")
                nc.vector.tensor_tensor(out=n2, in0=n1[:, 0:W // 4],
                                        in1=n1[:, W // 4:W // 2], op=OP.min)
                nc.vector.tensor_reduce(out=rowcol, in_=n2, axis=AX.X,
                                        op=OP.min)

            def emit_block_f32chain(d2, i):
                if d2 == 0:
                    LT, RT, lo = XS, YS, 0
                else:
                    LT, RT, lo = YS, XS, 32
                lc, lof = (i * P) // W, (i * P) % W
                lhsT = LT[lc][lo:lo + 24, lof:lof + P]
                rowcol = mins_all[:, h * 2 * NT + d2 * NT + i:
                                  h * 2 * NT + d2 * NT + i + 1]
                mm = work.tile([P, W], F32, tag="chainmm")
                mm16 = work.tile([P, W], FP16, tag="chainmm16")
                for g in range(NST):
                    ps = pstripe.tile([P, W], F32, tag="stripe")
                    for m in range(NMM):
                        nc.tensor.matmul(
                            ps[:, m * 512:(m + 1) * 512],
                            lhsT=lhsT,
                            rhs=RT[g][lo:lo + 24,
                                      m * 512:(m + 1) * 512],
                            start=True, stop=True)
                    if g == 0:
                        nc.scalar.copy(out=mm, in_=ps)
                    else:
                        # last level writes fp16 so the tail can use
                        # 2x-mode fp16 TT narrowing instead of a 1x reduce
                        dst = mm16 if g == NST - 1 else mm
                        nc.vector.tensor_tensor(out=dst, in0=ps, in1=mm,
                                                op=OP.min)
                if NST > 1:
                    c1 = work.tile([P, W // 2], FP16, tag="chainn1")
                    nc.vector.tensor_tensor(out=c1, in0=mm16[:, 0:W // 2],
                                            in1=mm16[:, W // 2:W], op=OP.min)
                    c2 = work.tile([P, W // 4], FP16, tag="chainn2")
                    nc.vector.tensor_tensor(out=c2, in0=c1[:, 0:W // 4],
                                            in1=c1[:, W // 4:W // 2], op=OP.min)
                    nc.vector.tensor_reduce(out=rowcol, in_=c2, axis=AX.X,
                                            op=OP.min)
                else:
                    nc.vector.tensor_reduce(out=rowcol, in_=mm, axis=AX.X,
                                            op=OP.min)

            def emit_block_unpaired(d2, i):
                if d2 == 0:
                    LT, RT, lo = XS, YS, 0
                else:
                    LT, RT, lo = YS, XS, 32
                lc, lof = (i * P) // W, (i * P) % W
                lhsT = LT[lc][lo:lo + 24, lof:lof + P]
                rowcol = mins_all[:, h * 2 * NT + d2 * NT + i:
                                  h * 2 * NT + d2 * NT + i + 1]
                ps = pstripe.tile([P, W], F32, tag="stripe")
                for m in range(NMM):
                    nc.tensor.matmul(
                        ps[:, m * 512:(m + 1) * 512],
                        lhsT=lhsT,
                        rhs=RT[0][lo:lo + 24, m * 512:(m + 1) * 512],
                        start=True, stop=True)
                nc.vector.tensor_reduce(out=rowcol, in_=ps, axis=AX.X,
                                        op=OP.min)

            # fp32-chain on 3/7 of blocks balances ACT vs DVE
            for i in range(NT):
                for d2 in range(2):
                    if NST == 1:
                        emit_block_unpaired(d2, i)
                    elif ((2 * i + d2) * 3) % 7 < 3:
                        emit_block_f32chain(d2, i)
                    else:
                        emit_block_f16(d2, i)

        # ---- regularizer: reg = sqrt(sum((Nhat Nhat^T - I)^2)), computed
        # redundantly across partitions with exact DVE ops.
        gsq = work.tile([P, 9], F32, tag="gsq")
        gtmp = work.tile([P, 3], F32, tag="gtmp")
        for m in range(3):
            for nn_ in range(3):
                nc.vector.tensor_tensor(out=gtmp, in0=nhat[:, m, :], in1=nhat[:, nn_, :],
                                        op=OP.mult)
                g1 = gsq[:, 3 * m + nn_:3 * m + nn_ + 1]
                nc.vector.tensor_reduce(out=g1, in_=gtmp, axis=AX.X, op=OP.add)
                if m == nn_:
                    nc.vector.tensor_scalar(out=g1, in0=g1, scalar1=-1.0,
                                            scalar2=None, op0=OP.add)
        nc.vector.tensor_tensor(out=gsq, in0=gsq, in1=gsq, op=OP.mult)
        q = work.tile([P, 1], F32, tag="q")
        nc.vector.tensor_reduce(out=q, in_=gsq, axis=AX.X, op=OP.add)
        sq0 = work.tile([P, 1], F32, tag="sq0")
        nc.scalar.activation(out=sq0, in_=q, func=AF.Sqrt)
        # Newton polish: sqrt = 0.5*(sq0 + q/sq0)
        rcp = work.tile([P, 1], F32, tag="rcp")
        nc.vector.reciprocal(out=rcp, in_=sq0)
        nc.vector.tensor_tensor(out=rcp, in0=rcp, in1=q, op=OP.mult)
        nc.vector.tensor_tensor(out=rcp, in0=rcp, in1=sq0, op=OP.add)
        reg = work.tile([P, 1], F32, tag="reg")
        nc.vector.tensor_scalar(out=reg, in0=rcp, scalar1=0.5 * REG_COEF,
                                scalar2=None, op0=OP.mult)

        # ---- final: sum(mins_all) over free dim, fold partitions, add reg
        sv = work.tile([P, 1], F32, tag="sv")
        nc.vector.tensor_reduce(out=sv, in_=mins_all, axis=AX.X, op=OP.add)
        # partition column -> single-partition row (exact, via DMA), then reduce
        row = work.tile([1, P], F32, tag="foldrow")
        nc.sync.dma_start(out=row, in_=sv)
        tot = work.tile([1, 1], F32, tag="tot")
        nc.vector.tensor_reduce(out=tot, in_=row, axis=AX.X, op=OP.add)
        final = work.tile([1, 1], F32, tag="final")
        nc.vector.tensor_tensor(out=final, in0=tot, in1=reg[0:1, :], op=OP.add)
        nc.sync.dma_start(out=out, in_=final)


_CACHE = {}


def _get_nc(n=4096):
    if n not in _CACHE:
        nc = bacc.Bacc("TRN2", target_bir_lowering=False, debug=False,
                       num_devices=B)
        emit_chamfer(nc, n)
        nc.compile()
        _CACHE[n] = nc
    return _CACHE[n]


def kernel(sample_points: np.ndarray, y_pred: np.ndarray) -> np.ndarray:
    assert sample_points.shape == (B, 4096, 3)
    assert y_pred.shape == (B, H, 4)
    nc = _get_nc(4096)
    in_maps = [
        {"pts": np.ascontiguousarray(sample_points[b], dtype=np.float32),
         "yp": np.ascontiguousarray(y_pred[b], dtype=np.float32)}
        for b in range(B)
    ]
    # the axon-tunneled device pool occasionally reports a transiently
    # wedged core; retry a few times before giving up
    import time as _time
    last_err = None
    for attempt in range(4):
        try:
            res = run_bass_kernel_spmd(nc, in_maps, list(range(B)))
            break
        except Exception as e:  # noqa: BLE001
            last_err = e
            _time.sleep(3.0 * (attempt + 1))
    else:
        raise last_err
    total = np.float64(0.0)
    for b in range(B):
        total += np.float64(res.results[b]["out"][0, 0])
    return np.asarray(total, dtype=np.float32).reshape(())



# revision 14
# speedup vs baseline: 2.5646x; 2.5646x over previous
"""Trainium2 Bass kernel for nn_ChamferLoss (reflection-symmetry chamfer loss).

Sharding: pure data parallel - batch b -> core b (B=8, 8 cores). Each core
computes its batch's loss; the host sums the 8 scalar partials.

Key algebraic identity: the reflection R_h is an isometric involution, so the
distance matrix d[i,j] = |x_i - R_h x_j|^2 is SYMMETRIC (d = d^T). Hence
sum_i min_j d + sum_j min_i d = 2 * sum_i min_j d - only ONE direction is
computed (48M distances instead of 96M).

Distance matmul: d[i,j] = sx_i + sy_j + u_i.y_j (u = -2x) on the PE with fp32
operands decomposed into 3 bf16 levels (6 kept cross products) stacked along
K=24 -> full-speed bf16 matmul at ~1e-6 accuracy. Aug tiles are built in
[128, NT, 32] point-major layout and transposed to matmul orientation with
the DMA xbar transpose (SBUF->SBUF, zero cost on compute engines); the X side
is used 4-chunk-stacked (base partitions 0/32/64/96), the Y side is
de-stacked to [32, 4096] rhs layout with 4 strided DMAs.

Min-reduction is split across THREE engines, balanced by a small LP:
- D-blocks (majority): ACT evacuates 2 PSUM stripes -> fp32 SBUF; DVE fuses
  min(PSUM, evac'd) for the other 2 stripes; the GPSIMD/Pool engine does the
  final rowmin with a free-axis MIN_INT tensor_reduce on the fp32 bits
  (IEEE order == int order for non-negative values; emitted as a raw
  InstTensorReduce since the cayman ISA allows it but the bass helper only
  exposes partition-axis reduces on Pool).
- A2-blocks: ACT evacuates all 4 stripes -> fp16; DVE runs a 2x-mode fp16
  min tree; final 128-wide partials are batched 8 blocks at a time into one
  DVE tensor_reduce.
- B2-blocks: ACT evacuates all 4 stripes -> fp32; Pool reduces all 4096.
Cost-model timeline ~237 us/core (vs 713 us for the both-directions
DVE/ACT-only baseline).
"""

import sys

sys.path.insert(0, "/opt/trn_rl_repo")

from contextlib import ExitStack

import numpy as np

import concourse.bass as bass
import concourse.bacc as bacc
import concourse.tile as tile
from concourse import mybir
from concourse.masks import make_identity
from concourse.bass_utils import run_bass_kernel_spmd

F32 = mybir.dt.float32
BF16 = mybir.dt.bfloat16
FP16 = mybir.dt.float16
I32 = mybir.dt.int32
AX = mybir.AxisListType
OP = mybir.AluOpType
AF = mybir.ActivationFunctionType

P = 128
H = 3
REG_COEF = 25.0
B = 8

# level patterns for the 6 kept cross products (x-level, y-level):
# (h,h) (h,m) (h,l) (m,h) (m,m) (l,h)
L_LEVELS = [0, 0, 0, 1, 1, 2]  # x-side level per 3-row group
R_LEVELS = [0, 1, 2, 0, 1, 0]  # y-side level per 3-row group

# per-block reduction recipe mix (96 blocks total), from the engine-balance LP
N_A2 = 12
N_B2 = 0


def _recipe_pattern():
    # uniform drain: every block does 2 ACT evacs + 2 DVE fused TTs; blocks
    # differ only in post-SBUF work (A2: DVE fp16 tree; D: Pool int-reduce).
    pat = ["D"] * 96
    for j in range(N_A2):
        pat[int((j + 0.5) / N_A2 * 96)] = "A2"
    placed = 0
    i = 3
    while placed < N_B2:
        if pat[i] == "D":
            pat[i] = "B2"
            placed += 1
        i += 37
    return pat


def _split3(nc, pool, src, shape, tag):
    """3-level bf16 split of an f32 tile: src ~= b0+b1+b2 (rel ~2^-25)."""
    outs = []
    cur = src
    for lv in range(3):
        b = pool.tile(shape, BF16, tag=f"{tag}b{lv}")
        nc.scalar.copy(out=b, in_=cur)
        outs.append(b)
        if lv < 2:
            r = pool.tile(shape, F32, tag=f"{tag}r{lv}")
            nc.vector.tensor_tensor(out=r, in0=cur, in1=b, op=OP.subtract)
            cur = r
    return outs


def _pool_reduce_min_i32(nc, out_col_f32, in_f32):
    """Free-axis MIN_INT tensor_reduce on the Pool engine over fp32 bits."""
    g = nc.gpsimd
    ini = in_f32.bitcast(I32)
    outi = out_col_f32.bitcast(I32)
    return g.add_instruction(mybir.InstTensorReduce(
        name=f"I-{g.bass.next_id()}",
        op=OP.min, axis=AX.X,
        ins=[g.lower_ap(ini.opt(keep_dims=frozenset({0, len(ini.shape) - 1})),
                        opt=False)],
        outs=[g.lower_ap(outi)],
        apply_absolute_value=None, apply_transpose=None, negate=None))


def emit_chamfer(nc, n=4096):
    NT = n // P           # 32 point chunks of 128
    NQ = NT // 4          # 4-chunk transpose groups

    pts = nc.dram_tensor("pts", [n, 3], F32, kind="ExternalInput").ap()
    yp = nc.dram_tensor("yp", [H, 4], F32, kind="ExternalInput").ap()
    out = nc.dram_tensor("out", [1, 1], F32, kind="ExternalOutput").ap()

    with ExitStack() as ctx:
        tc = ctx.enter_context(tile.TileContext(nc))
        const = ctx.enter_context(tc.tile_pool(name="const", bufs=1))
        work = ctx.enter_context(tc.tile_pool(name="work", bufs=2))
        headp = ctx.enter_context(tc.tile_pool(name="headp", bufs=2))
        sb = ctx.enter_context(tc.tile_pool(name="sb", bufs=3))
        pstripe = ctx.enter_context(tc.tile_pool(
            name="pstripe", bufs=4, space="PSUM"))

        # ---- load points: Xn[p, t, c] = pts[t*128+p, c]
        Xn = const.tile([P, NT, 3], F32)
        nc.sync.dma_start(out=Xn, in_=pts.rearrange("(t p) c -> p t c", p=P))

        # ---- yp broadcast to all partitions
        ypb = const.tile([P, H, 4], F32)
        yp_b = bass.AP(tensor=yp.tensor, offset=yp.offset,
                       ap=[[0, P], [4, H], [1, 4]])
        nc.sync.dma_start(out=ypb, in_=yp_b)

        # ---- sx = |x|^2 per point
        Xsq = work.tile([P, NT, 3], F32)
        nc.vector.tensor_tensor(out=Xsq, in0=Xn, in1=Xn, op=OP.mult)
        sx = const.tile([P, NT], F32)
        nc.vector.tensor_tensor(out=sx, in0=Xsq[:, :, 0], in1=Xsq[:, :, 1],
                                op=OP.add)
        nc.vector.tensor_tensor(out=sx, in0=sx, in1=Xsq[:, :, 2], op=OP.add)

        # ---- u = -2x splits and sx splits
        U = work.tile([P, NT, 3], F32)
        nc.vector.tensor_scalar(out=U, in0=Xn, scalar1=-2.0, scalar2=None,
                                op0=OP.mult)
        ub = _split3(nc, work, U, [P, NT, 3], "u")
        sxb = _split3(nc, work, sx, [P, NT], "sx")

        # ---- X aug [P, NT, 32]: rows 0-17 u levels, 18-20 sx splits,
        # 21-23 ones, 24-31 zero pad; xbar-transpose to 4-chunk-stacked
        # XT4 [128, NQ*128] (chunk t rows at partitions (t%4)*32..+23)
        XSa = const.tile([P, NT, 32], BF16)
        nc.gpsimd.memset(XSa[:, :, 21:24], 1.0)
        nc.gpsimd.memset(XSa[:, :, 24:32], 0.0)
        for g, lv in enumerate(L_LEVELS):
            nc.gpsimd.tensor_copy(out=XSa[:, :, 3 * g:3 * g + 3], in_=ub[lv])
        for l in range(3):
            nc.gpsimd.tensor_copy(out=XSa[:, :, 18 + l], in_=sxb[l])
        id128 = const.tile([P, P], BF16)
        make_identity(nc, id128)

        def pe_transpose_side(aug, dest, groups=None):
            # startup path: PE transposes via PSUM + ACT evac (PE/PSUM idle
            # here; avoids the serialized DMA-engine chain at kernel start)
            for qq in (groups if groups is not None else range(NT // 8)):
                pt = pstripe.tile([32, 8 * P], BF16, tag="stripe", name="pt")
                for j in range(8):
                    t = qq * 8 + j
                    nc.tensor.transpose(pt[:, j * P:(j + 1) * P],
                                        aug[:, t, :], id128)
                nc.scalar.copy(out=dest[:, qq * 8 * P:(qq + 1) * 8 * P],
                               in_=pt)

        xt = const.tile([32, n], BF16)
        pe_transpose_side(XSa, xt)

        # ---- per-head Y sides
        nhat = const.tile([P, H, 3], F32)
        YT = {}

        def emit_head_setup(h):
            # normalize head normal (exact DVE ops + ACT sqrt + Newton)
            sqn = headp.tile([P, 3], F32, tag="sqn")
            nc.vector.tensor_tensor(out=sqn, in0=ypb[:, h, 0:3],
                                    in1=ypb[:, h, 0:3], op=OP.mult)
            nn = headp.tile([P, 1], F32, tag="nn")
            nc.vector.tensor_reduce(out=nn, in_=sqn, axis=AX.X, op=OP.add)
            sq_ = headp.tile([P, 1], F32, tag="sq_")
            nc.scalar.activation(out=sq_, in_=nn, func=AF.Sqrt)
            rs0 = headp.tile([P, 1], F32, tag="rs0")
            nc.vector.reciprocal(out=rs0, in_=sq_)
            a = headp.tile([P, 1], F32, tag="nta")
            nc.vector.tensor_tensor(out=a, in0=rs0, in1=rs0, op=OP.mult)
            nc.vector.tensor_tensor(out=a, in0=a, in1=nn, op=OP.mult)
            nc.vector.tensor_scalar(out=a, in0=a, scalar1=-0.5, scalar2=1.5,
                                    op0=OP.mult, op1=OP.add)
            rs = headp.tile([P, 1], F32, tag="rs")
            nc.vector.tensor_tensor(out=rs, in0=rs0, in1=a, op=OP.mult)
            nc.vector.tensor_scalar(out=nhat[:, h, :], in0=ypb[:, h, 0:3],
                                    scalar1=rs, scalar2=None, op0=OP.mult)
            off = ypb[:, h, 3:4]

            # s[p,t] = nhat . x + off
            s = headp.tile([P, NT], F32, tag="s")
            t0 = headp.tile([P, NT], F32, tag="t0")
            nc.vector.tensor_scalar(out=s, in0=Xn[:, :, 0],
                                    scalar1=nhat[:, h, 0:1], scalar2=off,
                                    op0=OP.mult, op1=OP.add)
            nc.vector.tensor_scalar(out=t0, in0=Xn[:, :, 1],
                                    scalar1=nhat[:, h, 1:2], scalar2=None,
                                    op0=OP.mult)
            nc.vector.tensor_tensor(out=s, in0=s, in1=t0, op=OP.add)
            nc.vector.tensor_scalar(out=t0, in0=Xn[:, :, 2],
                                    scalar1=nhat[:, h, 2:3], scalar2=None,
                                    op0=OP.mult)
            nc.vector.tensor_tensor(out=s, in0=s, in1=t0, op=OP.add)

            # reflected points Yn = x - 2 s nhat ; sy = sx + 4*off*s
            m2 = headp.tile([P, 3], F32, tag="m2")
            nc.vector.tensor_scalar(out=m2, in0=nhat[:, h, :], scalar1=-2.0,
                                    scalar2=None, op0=OP.mult)
            Yn = headp.tile([P, NT, 3], F32, tag="Yn")
            tc_ = headp.tile([P, NT], F32, tag="tc_")
            for c in range(3):
                nc.vector.tensor_scalar(out=tc_, in0=s, scalar1=m2[:, c:c + 1],
                                        scalar2=None, op0=OP.mult)
                nc.vector.tensor_tensor(out=Yn[:, :, c], in0=Xn[:, :, c],
                                        in1=tc_, op=OP.add)
            o4 = headp.tile([P, 1], F32, tag="o4")
            nc.vector.tensor_scalar(out=o4, in0=off, scalar1=4.0, scalar2=None,
                                    op0=OP.mult)
            sy = headp.tile([P, NT], F32, tag="sy")
            nc.vector.tensor_scalar(out=sy, in0=s, scalar1=o4, scalar2=None,
                                    op0=OP.mult)
            nc.vector.tensor_tensor(out=sy, in0=sy, in1=sx, op=OP.add)

            # y / sy splits and Y aug [P, NT, 32]:
            # rows 0-17 y levels R, 18-20 ones, 21-23 sy splits, 24-31 pad
            yb = _split3(nc, headp, Yn, [P, NT, 3], "y")
            syb = _split3(nc, headp, sy, [P, NT], "sy")
            YSa = headp.tile([P, NT, 32], BF16, tag="YSa")
            cpe = nc.gpsimd if h == 0 else nc.scalar
            nc.gpsimd.memset(YSa[:, :, 18:21], 1.0)
            nc.gpsimd.memset(YSa[:, :, 24:32], 0.0)
            for g, lv in enumerate(R_LEVELS):
                cpe.tensor_copy(out=YSa[:, :, 3 * g:3 * g + 3], in_=yb[lv])
            for l in range(3):
                cpe.tensor_copy(out=YSa[:, :, 21 + l], in_=syb[l])

            # transpose to rhs layout [32, n]: PE route for head 0 (fast
            # startup), xbar DMA route for heads 1/2 (off compute engines)
            yt = const.tile([32, n], BF16, tag=f"yt{h}")
            if h == 0:
                pe_transpose_side(YSa, yt)
            else:
                YT4 = headp.tile([P, NQ * P], BF16, tag="YT4")
                for q in range(NQ):
                    nc.sync.dma_start_transpose(
                        out=YT4[:, q * P:(q + 1) * P],
                        in_=YSa[:, 4 * q:4 * q + 4, :])
                ytv = yt.rearrange("r (q k p) -> r q k p", k=4, p=P)
                y4v = YT4.rearrange("r (q p) -> r q p", p=P)
                for k in range(4):
                    nc.sync.dma_start(out=ytv[:, :, k, :],
                                      in_=y4v[32 * k:32 * k + 32, :, :])
            YT[h] = yt

        def emit_reg(reg):
            # regularizer: needs all heads' nhat (exact DVE ops)
            gsq = work.tile([P, 9], F32, tag="gsq")
            gtmp = work.tile([P, 3], F32, tag="gtmp")
            for m in range(3):
                for nn_ in range(3):
                    nc.vector.tensor_tensor(out=gtmp, in0=nhat[:, m, :],
                                            in1=nhat[:, nn_, :], op=OP.mult)
                    g1 = gsq[:, 3 * m + nn_:3 * m + nn_ + 1]
                    nc.vector.tensor_reduce(out=g1, in_=gtmp, axis=AX.X,
                                            op=OP.add)
                    if m == nn_:
                        nc.vector.tensor_scalar(out=g1, in0=g1, scalar1=-1.0,
                                                scalar2=None, op0=OP.add)
            nc.vector.tensor_tensor(out=gsq, in0=gsq, in1=gsq, op=OP.mult)
            q = work.tile([P, 1], F32, tag="q")
            nc.vector.tensor_reduce(out=q, in_=gsq, axis=AX.X, op=OP.add)
            sq0 = work.tile([P, 1], F32, tag="sq0")
            nc.scalar.activation(out=sq0, in_=q, func=AF.Sqrt)
            rcp = work.tile([P, 1], F32, tag="rcp")
            nc.vector.reciprocal(out=rcp, in_=sq0)
            nc.vector.tensor_tensor(out=rcp, in0=rcp, in1=q, op=OP.mult)
            nc.vector.tensor_tensor(out=rcp, in0=rcp, in1=sq0, op=OP.add)
            nc.vector.tensor_scalar(out=reg, in0=rcp, scalar1=0.5 * REG_COEF,
                                    scalar2=None, op0=OP.mult)

        # ---- main loop: 96 (head, row-block) blocks, one direction only.
        # Head h+1's setup (and the regularizer) are emitted a few blocks
        # into head h's stream so engine FIFOs overlap setup with reduction.
        mins_all = const.tile([P, 2 * NT * H], F32)  # cols 96.. unused
        pattern = _recipe_pattern()
        state = {"a2_idx": 0, "other_col": N_A2, "bt": None,
                 "bt_fill": 0, "bt_base": 0}
        reg = work.tile([P, 1], F32, tag="reg")
        emit_head_setup(0)

        def emit_block(h, i, recipe, st):
                lhsT = xt[0:24, i * P:(i + 1) * P]
                ss = []
                for g in range(4):
                    ps = pstripe.tile([P, 1024], F32, tag="stripe")
                    for m in range(2):
                        nc.tensor.matmul(
                            ps[:, m * 512:(m + 1) * 512],
                            lhsT=lhsT,
                            rhs=YT[h][0:24, g * 1024 + m * 512:
                                      g * 1024 + (m + 1) * 512],
                            start=True, stop=True)
                    ss.append(ps)

                if recipe == "D":
                    e32 = sb.tile([P, 2048], F32, tag="e32", bufs=4)
                    nc.scalar.copy(out=e32[:, 0:1024], in_=ss[0])
                    nc.scalar.copy(out=e32[:, 1024:2048], in_=ss[1])
                    m32 = sb.tile([P, 2048], F32, tag="m32", bufs=6)
                    nc.vector.tensor_tensor(out=m32[:, 0:1024], in0=ss[2],
                                            in1=e32[:, 0:1024], op=OP.min)
                    nc.vector.tensor_tensor(out=m32[:, 1024:2048], in0=ss[3],
                                            in1=e32[:, 1024:2048], op=OP.min)
                    _pool_reduce_min_i32(
                        nc, mins_all[:, st["other_col"]:st["other_col"] + 1],
                        m32)
                    st["other_col"] += 1
                elif recipe == "B2":
                    e4 = sb.tile([P, 4096], F32, tag="e4k", bufs=1)
                    for g in range(4):
                        nc.scalar.copy(out=e4[:, g * 1024:(g + 1) * 1024],
                                       in_=ss[g])
                    _pool_reduce_min_i32(
                        nc, mins_all[:, st["other_col"]:st["other_col"] + 1],
                        e4)
                    st["other_col"] += 1
                else:  # A2: same drain shape as D, fp16 + DVE tree tail
                    e16 = sb.tile([P, 2048], FP16, tag="e16")
                    nc.scalar.copy(out=e16[:, 0:1024], in_=ss[0])
                    nc.scalar.copy(out=e16[:, 1024:2048], in_=ss[1])
                    m16 = sb.tile([P, 2048], FP16, tag="m16")
                    nc.vector.tensor_tensor(out=m16[:, 0:1024], in0=ss[2],
                                            in1=e16[:, 0:1024], op=OP.min)
                    nc.vector.tensor_tensor(out=m16[:, 1024:2048], in0=ss[3],
                                            in1=e16[:, 1024:2048], op=OP.min)
                    c1 = sb.tile([P, 1024], FP16, tag="c1")
                    nc.vector.tensor_tensor(out=c1, in0=m16[:, 0:1024],
                                            in1=m16[:, 1024:2048], op=OP.min)
                    c2 = sb.tile([P, 512], FP16, tag="c2")
                    nc.vector.tensor_tensor(out=c2, in0=c1[:, 0:512],
                                            in1=c1[:, 512:1024], op=OP.min)
                    c3 = sb.tile([P, 256], FP16, tag="c3")
                    nc.vector.tensor_tensor(out=c3, in0=c2[:, 0:256],
                                            in1=c2[:, 256:512], op=OP.min)
                    if st["bt"] is None:
                        st["bt"] = sb.tile([P, 8, P], F32, tag="bt", bufs=2, name="bt")
                        st["bt_fill"] = 0
                        st["bt_base"] = st["a2_idx"]
                    nc.vector.tensor_tensor(out=st["bt"][:, st["bt_fill"], :],
                                            in0=c3[:, 0:128],
                                            in1=c3[:, 128:256], op=OP.min)
                    st["bt_fill"] += 1
                    st["a2_idx"] += 1
                    if st["bt_fill"] == 8 or st["a2_idx"] == N_A2:
                        nc.vector.tensor_reduce(
                            out=mins_all[:, st["bt_base"]:
                                         st["bt_base"] + st["bt_fill"]],
                            in_=st["bt"][:, 0:st["bt_fill"], :],
                            axis=AX.X, op=OP.min)
                        st["bt"] = None

        bidx = 0
        for h in range(H):
            for i in range(NT):
                emit_block(h, i, pattern[bidx], state)
                bidx += 1
                if h < H - 1 and i == 5:
                    emit_head_setup(h + 1)
                if h == H - 1 and i == 0:
                    emit_reg(reg)

        # ---- final: 2 * sum(rowmins) + reg
        sv = work.tile([P, 1], F32, tag="sv")
        nc.vector.tensor_reduce(out=sv, in_=mins_all[:, 0:96], axis=AX.X,
                                op=OP.add)
        row = work.tile([1, P], F32, tag="foldrow")
        nc.sync.dma_start(out=row, in_=sv)
        tot = work.tile([1, 1], F32, tag="tot")
        nc.vector.tensor_reduce(out=tot, in_=row, axis=AX.X, op=OP.add)
        final = work.tile([1, 1], F32, tag="final")
        nc.vector.tensor_scalar(out=final, in0=tot, scalar1=2.0, scalar2=None,
                                op0=OP.mult)
        nc.vector.tensor_tensor(out=final, in0=final, in1=reg[0:1, :],
                                op=OP.add)
        nc.sync.dma_start(out=out, in_=final)


_CACHE = {}


def _get_nc(n=4096):
    if n not in _CACHE:
        nc = bacc.Bacc("TRN2", target_bir_lowering=False, debug=False,
                       num_devices=B)
        emit_chamfer(nc, n)
        nc.compile()
        _CACHE[n] = nc
    return _CACHE[n]


def kernel(sample_points: np.ndarray, y_pred: np.ndarray) -> np.ndarray:
    assert sample_points.shape == (B, 4096, 3)
    assert y_pred.shape == (B, H, 4)
    nc = _get_nc(4096)
    in_maps = [
        {"pts": np.ascontiguousarray(sample_points[b], dtype=np.float32),
         "yp": np.ascontiguousarray(y_pred[b], dtype=np.float32)}
        for b in range(B)
    ]
    # the axon-tunneled device pool occasionally reports a transiently
    # wedged core; retry a few times before giving up
    import time as _time
    last_err = None
    for attempt in range(4):
        try:
            res = run_bass_kernel_spmd(nc, in_maps, list(range(B)))
            break
        except Exception as e:  # noqa: BLE001
            last_err = e
            _time.sleep(3.0 * (attempt + 1))
    else:
        raise last_err
    total = np.float64(0.0)
    for b in range(B):
        total += np.float64(res.results[b]["out"][0, 0])
    return np.asarray(total, dtype=np.float32).reshape(())
